# revision 24
# baseline (speedup 1.0000x reference)
"""One-pole IIR filter (DOnePole) on 8 Trainium2 NeuronCores.

Reference semantics (per batch element b, scan over time t):
    out_t = b0*x_t + s_t ;  s_{t+1} = b1*x_t + a*out_t   (a = clip(a1,-1,1))
i.e. out = x convolved with the causal kernel
    h_0 = b0,  h_k = (b0*a + b1) * a^(k-1)  for k >= 1.

Fast path (|a| <= 0.95): the kernel decays below fp16 resolution within a
couple hundred taps, so out is computed as a 255-tap truncated FIR entirely
on the tensor engine. The host re-packs each core's (32, 131072) slice
time-major as [128, 1024*32] fp16: partition p = t mod 128, free column
f = 32*m + b for time-block m = t div 128 and batch b. Then

    outT[:, (m,b)] = L0 @ xT[:, (m,b)] + L1 @ xT[:, (m-1,b)]

with host-built 128x128 Toeplitz matrices L0[j,p] = h[p-j] (p>=j) and
L1[j,p] = h[128+p-j]: two PE matmuls per 512-column PSUM chunk (fp32
accumulate), ACT/DVE copy the chunk back to fp16 SBUF, store. The
(m-1,b) operand is just the same SBUF tile shifted 32 columns, so tiles
carry a 32-column left margin (zero for the first tile: s_0 = 0). All
I/O is fp16 (the rel-err budget is 2e-2; fp16 costs ~3e-4, and for
a = 0.5 every h_k is a power of two, exact in fp16), which halves HBM
traffic - the hard per-core limit (~358 GB/s) that bounds this kernel.

Fallback (|a| > 0.95): the original fp32 tensor_tensor_scan kernel with
matmul segment stitching (exact for any a, ~2.8x slower).

Distribution: data-parallel over batch, 32 rows per core, both paths.
"""

import sys
from contextlib import ExitStack
from functools import lru_cache

import numpy as np

sys.path.insert(0, "/opt/trn_rl_repo")

import concourse.bass as bass  # noqa: E402
import concourse.tile as tile  # noqa: E402
from concourse import bacc, mybir  # noqa: E402
from concourse.bass_utils import run_bass_kernel_spmd  # noqa: E402

N_CORES = 8
B_FULL, T_FULL = 256, 131072
B_LOC = B_FULL // N_CORES          # 32 batch rows per core
P = 128                            # time-block size = SBUF partitions
M_BLK = T_FULL // P                # 1024 time blocks per batch row
W = M_BLK * B_LOC                  # 32768 free columns per core
MARGIN = B_LOC                     # 32-column shift = one time block
CHUNK = 512                        # matmul moving-operand max / PSUM bank
FP32 = mybir.dt.float32
FP16 = mybir.dt.float16

# Debug knobs (used by the local test harness only; harmless defaults).
TRACE = False
TRACE_DIR = None
LAST_RESULT = None


def _h_coeffs(a: float, b0: float, b1: float, n: int) -> np.ndarray:
    """Impulse response h_0..h_{n-1} of the filter, float64."""
    h = np.zeros(n, dtype=np.float64)
    h[0] = b0
    if n > 1:
        c = b0 * a + b1
        h[1:] = c * np.float64(a) ** np.arange(n - 1, dtype=np.float64)
    return h


def _mm_consts(a: float, b0: float, b1: float):
    """L0[j,p] = h[p-j] (p>=j), L1[j,p] = h[128+p-j]; fp16 [128,128]."""
    h = _h_coeffs(a, b0, b1, 2 * P)
    j = np.arange(P)[:, None]
    p = np.arange(P)[None, :]
    l0 = np.where(p >= j, h[np.minimum(p - j, 2 * P - 1)], 0.0)
    l1 = h[128 + p - j]
    return (
        np.ascontiguousarray(l0.astype(np.float16)),
        np.ascontiguousarray(l1.astype(np.float16)),
    )


def _mm_load_widths():
    """Graduated load slices: small first loads cut time-to-first-matmul;
    later loads are big so their DMA descriptors (per-partition bytes) stay
    large - SDMA round-robins between the load and store queues at PACKET
    granularity, so small descriptors starve against 8KB store descriptors.
    Multiples of 2048; sum = W."""
    return [1024, 1024, 2048, 4096, 4096, 8192, 8192, 4096]


STORE_W = 4096                     # uniform 1MB store slices


def _build_mm_program():
    """Truncated-FIR path: the filter is folded into host-built matrices,
    so one compiled program serves every (a, b0, b1) on this path.

    All SBUF buffers are statically allocated (input 8.5MB + output 8.4MB
    fit simultaneously), so no stage ever stalls on buffer reuse: loads all
    issue up front, stores stream out as soon as each 4096-col slice is
    copied from PSUM."""
    lws = _mm_load_widths()
    assert sum(lws) == W
    l_offs = np.concatenate([[0], np.cumsum(lws)]).astype(int)

    nc = bacc.Bacc("TRN2", target_bir_lowering=False, debug=False)

    x = nc.dram_tensor("x", [P, W], FP16, kind="ExternalInput")
    l0 = nc.dram_tensor("l0", [P, P], FP16, kind="ExternalInput")
    l1 = nc.dram_tensor("l1", [P, P], FP16, kind="ExternalInput")
    out = nc.dram_tensor("out", [P, W], FP16, kind="ExternalOutput")

    with tile.TileContext(nc) as tc, ExitStack() as ctx:
        wpool = ctx.enter_context(tc.tile_pool(name="wpool", bufs=1))
        xpool = ctx.enter_context(tc.tile_pool(name="xpool", bufs=1))
        opool = ctx.enter_context(tc.tile_pool(name="opool", bufs=1))
        pspool = ctx.enter_context(tc.tile_pool(name="pspool", bufs=8, space="PSUM"))

        # weights ride the (otherwise idle at t=0) ACT ring
        l0_sb = wpool.tile([P, P], FP16)
        nc.scalar.dma_start(l0_sb[:], l0[:])
        l1_sb = wpool.tile([P, P], FP16)
        nc.scalar.dma_start(l1_sb[:], l1[:])

        # PE warm-up: the HAM clock gate holds the PE at 1.2 GHz until it has
        # seen a ~3.4us busy window. The PE is otherwise idle while the first
        # loads run, so burn that head on dummy matmuls over scratch SBUF -
        # the real matmuls then start at 2.4 GHz instead of ramping (the cold
        # ramp is also the main source of run-to-run variance).
        scr = wpool.tile([P, CHUNK], FP16)
        nc.gpsimd.memset(scr[:], 0.0)
        ps_warm = pspool.tile([P, CHUNK], FP32, tag="ps", name="ps_warm")
        for k in range(8):
            nc.tensor.matmul(
                ps_warm[:], scr[:, 0:P], scr[:], start=True, stop=True
            )

        # all input tiles (one per load slice, no reuse), loads back-to-back
        xts = []
        for i, w in enumerate(lws):
            lo = int(l_offs[i])
            xt = xpool.tile([P, MARGIN + w], FP16, tag=f"xt_{i}")
            if i == 0:
                nc.vector.memset(xt[:, 0:MARGIN], 0.0)
                nc.sync.dma_start(xt[:, MARGIN : MARGIN + w], x[:, 0:w])
            else:
                nc.sync.dma_start(xt[:, 0 : MARGIN + w], x[:, lo - MARGIN : lo + w])
            xts.append(xt)

        # store slices: uniform 1MB, tapered at the end so the final
        # copy+store tail is short
        store_ws = [4096] * 7 + [2048, 1024, 1024]
        assert sum(store_ws) == W
        grp = 0
        slo = 0
        for s, sw in enumerate(store_ws):
            ot = opool.tile([P, sw], FP16, tag=f"ot_{s}")
            # weight-groups: runs of up to 4 chunks within one load tile
            # share one LDWEIGHTS per matrix (PE reloads weights per matmul
            # group otherwise, ~100ns each)
            g = slo
            while g < slo + sw:
                ti = int(np.searchsorted(l_offs, g, side="right")) - 1
                run_end = min(slo + sw, int(l_offs[ti + 1]), g + 4 * CHUNK)
                loc = g - int(l_offs[ti])
                xt = xts[ti]
                n_ch = (run_end - g) // CHUNK
                pss = [
                    pspool.tile([P, CHUNK], FP32, tag="ps", name=f"ps_{grp}_{c}")
                    for c in range(n_ch)
                ]
                for c in range(n_ch):
                    nc.tensor.matmul(
                        pss[c][:],
                        l0_sb[:],
                        xt[:, MARGIN + loc + c * CHUNK : MARGIN + loc + (c + 1) * CHUNK],
                        start=True,
                        stop=False,
                    )
                for c in range(n_ch):
                    nc.tensor.matmul(
                        pss[c][:],
                        l1_sb[:],
                        xt[:, loc + c * CHUNK : loc + (c + 1) * CHUNK],
                        start=False,
                        stop=True,
                    )
                for c in range(n_ch):
                    osl = ot[:, g - slo + c * CHUNK : g - slo + (c + 1) * CHUNK]
                    if grp % 2 == 0:
                        nc.scalar.copy(osl, pss[c][:])
                    else:
                        nc.vector.tensor_copy(out=osl, in_=pss[c][:])
                    grp += 1
                g = run_end
            # split stores across both HWDGE rings: the sync ring is idle
            # once loads are dispatched, and a single store queue only
            # sustains ~260 GB/s
            seng = nc.scalar if s % 2 == 0 else nc.sync
            seng.dma_start(out[:, slo : slo + sw], ot[:])
            slo += sw

    nc.compile()
    return nc


# ---------------------------------------------------------------------------
# Fallback path (|a| > 0.95): fp32 tensor_tensor_scan + matmul stitching.
# ---------------------------------------------------------------------------

SEGS = 128 // B_LOC                # 4 time segments per batch row
T_SEG = T_FULL // SEGS             # 32768 columns per partition row


def _kfix(a: float) -> int:
    """Columns over which the a^t segment-stitch correction is applied."""
    aa = abs(a)
    if aa >= 1.0:
        return T_SEG
    if aa == 0.0:
        return 1
    return int(min(T_SEG, max(1, int(np.ceil(np.log(1e-14) / np.log(aa))))))


def _tile_widths():
    return [512, 1536, 2048, 4096, 4096, 4096, 4096, 4096, 4096, 2048, 2048]


def _build_program(a: float, b0: float, b1: float, kfix: int):
    widths = _tile_widths()
    offs = np.concatenate([[0], np.cumsum(widths)]).astype(int)
    n_t = len(widths)
    held_idx = [i for i in range(n_t) if offs[i] < kfix]

    nc = bacc.Bacc("TRN2", target_bir_lowering=False, debug=False)

    x = nc.dram_tensor("x", [128, T_SEG], FP32, kind="ExternalInput")
    ramp = nc.dram_tensor("ramp", [128, kfix], FP32, kind="ExternalInput")
    pmatT = nc.dram_tensor("pmatT", [128, 128], FP32, kind="ExternalInput")
    out = nc.dram_tensor("out", [128, T_SEG], FP32, kind="ExternalOutput")

    fast = (b1 == 0.0)
    ident = fast and (b0 == 1.0)

    with tile.TileContext(nc) as tc, ExitStack() as ctx:
        many_held = len(held_idx) > 8
        cpool = ctx.enter_context(tc.tile_pool(name="cpool", bufs=1))
        xpool = ctx.enter_context(
            tc.tile_pool(name="xpool", bufs=2 if many_held else 4)
        )
        hpool = ctx.enter_context(tc.tile_pool(name="hpool", bufs=1))
        spool = ctx.enter_context(tc.tile_pool(name="spool", bufs=1))
        pspool = ctx.enter_context(tc.tile_pool(name="pspool", bufs=1, space="PSUM"))
        opool = ctx.enter_context(
            tc.tile_pool(name="opool", bufs=2 if many_held else 3)
        )

        ac2k = cpool.tile([128, 2048], FP32)
        nc.gpsimd.memset(ac2k[:], a)
        ac4k = cpool.tile([128, 4096], FP32)
        nc.gpsimd.memset(ac4k[:], a)

        if not ident:
            zcol = spool.tile([128, 1], FP32)
            nc.gpsimd.memset(zcol[:], 0.0)

        held = {}
        prev_out = None
        prev_x = None
        prev_w = 0
        for i in range(n_t):
            w = widths[i]
            lo, hi = int(offs[i]), int(offs[i] + w)
            wide = w > 2048
            xt = xpool.tile(
                [128, 4096 if wide else 2048], FP32,
                tag="xtb" if wide else "xt", bufs=3 if wide else None,
                name=f"xt_{i}",
            )
            nc.sync.dma_start(xt[:, 0:w], x[:, lo:hi])

            if ident:
                data1 = xt
            else:
                ut = xpool.tile(
                    [128, 4096 if wide else 2048], FP32,
                    tag="utb" if wide else "ut", bufs=3 if wide else None,
                    name=f"ut_{i}",
                )
                nc.scalar.mul(ut[:, 0:w], xt[:, 0:w], b0)
                if not fast:
                    nc.vector.scalar_tensor_tensor(
                        out=ut[:, 1:w],
                        in0=xt[:, 0 : w - 1],
                        scalar=b1,
                        in1=ut[:, 1:w],
                        op0=mybir.AluOpType.mult,
                        op1=mybir.AluOpType.add,
                    )
                    xprev_col = (
                        zcol[:, 0:1] if i == 0 else prev_x[:, prev_w - 1 : prev_w]
                    )
                    nc.vector.scalar_tensor_tensor(
                        out=ut[:, 0:1],
                        in0=xprev_col,
                        scalar=b1,
                        in1=ut[:, 0:1],
                        op0=mybir.AluOpType.mult,
                        op1=mybir.AluOpType.add,
                    )
                data1 = ut

            is_held = i in held_idx
            ot = (hpool if is_held else opool).tile(
                [128, 4096 if wide else 2048], FP32,
                tag=(f"held{i}" if is_held else ("otb" if wide else "ot")),
                bufs=1 if is_held else None, name=f"ot_{i}",
            )
            init = 0.0 if i == 0 else prev_out[:, prev_w - 1 : prev_w]
            ac = ac4k if wide else ac2k
            nc.vector.tensor_tensor_scan(
                out=ot[:, 0:w],
                data0=ac[:, 0:w],
                data1=data1[:, 0:w],
                initial=init,
                op0=mybir.AluOpType.mult,
                op1=mybir.AluOpType.add,
            )
            if is_held:
                held[i] = ot
            else:
                seng = nc.sync if i >= n_t - 3 else nc.scalar
                seng.dma_start(out[:, lo:hi], ot[:, 0:w])
            prev_out = ot
            prev_x = xt
            prev_w = w

        pm_sb = spool.tile([128, 128], FP32)
        nc.scalar.dma_start(pm_sb[:], pmatT[:])

        d_t = spool.tile([128, 1], FP32)
        nc.vector.tensor_scalar_mul(d_t[:], prev_out[:, prev_w - 1 : prev_w], a)
        if not fast:
            nc.vector.scalar_tensor_tensor(
                out=d_t[:],
                in0=prev_x[:, prev_w - 1 : prev_w],
                scalar=b1,
                in1=d_t[:],
                op0=mybir.AluOpType.mult,
                op1=mybir.AluOpType.add,
            )

        s_ps = pspool.tile([128, 1], FP32)
        nc.tensor.matmul(s_ps[:], pm_sb[:], d_t[:], start=True, stop=True)
        s_sb = spool.tile([128, 1], FP32)
        nc.scalar.copy(s_sb[:], s_ps[:])

        for i in held_idx:
            ot = held[i]
            lo = int(offs[i])
            w = min(widths[i], kfix - lo)
            rt = xpool.tile(
                [128, min(kfix, widths[i])], FP32, tag="rt",
                bufs=1 if many_held else 2, name=f"rt_{i}",
            )
            nc.scalar.dma_start(rt[:, 0:w], ramp[:, lo : lo + w])
            nc.vector.scalar_tensor_tensor(
                out=ot[:, 0:w],
                in0=rt[:, 0:w],
                scalar=s_sb[:],
                in1=ot[:, 0:w],
                op0=mybir.AluOpType.mult,
                op1=mybir.AluOpType.add,
            )
            nc.sync.dma_start(out[:, lo : lo + widths[i]], ot[:, 0 : widths[i]])

    nc.compile()
    return nc


@lru_cache(maxsize=8)
def _get_mm_program():
    return _build_mm_program()


@lru_cache(maxsize=8)
def _get_program(a: float, b0: float, b1: float):
    """Program used for profiling hooks in the local harness: returns the
    matmul-path program when it applies, else the scan-path program."""
    if abs(a) <= 0.95:
        return _get_mm_program(), 0
    kfix = _kfix(a)
    return _build_program(a, b0, b1, kfix), kfix


def _host_consts(a: float, kfix: int):
    ramp = (np.float64(a) ** np.arange(kfix, dtype=np.float64)).astype(np.float32)
    ramp_b = np.ascontiguousarray(np.broadcast_to(ramp[None, :], (128, kfix)))
    aL = np.float64(a) ** np.float64(T_SEG)
    Pm = np.zeros((128, 128), dtype=np.float64)
    for b in range(B_LOC):
        for j in range(SEGS):
            for j2 in range(j):
                Pm[SEGS * b + j, SEGS * b + j2] = aL ** (j - j2 - 1)
    pmatT = np.ascontiguousarray(Pm.T.astype(np.float32))
    return ramp_b, pmatT


def _ensure_axon_hooks():
    """bass_utils imports antenv.axon_hooks when tracing is requested; some
    images lack that module. Provide a stub that reports 'no hook' so
    execution proceeds untraced instead of dying."""
    try:
        import antenv.axon_hooks  # noqa: F401
    except Exception:
        import types

        mod = types.ModuleType("antenv.axon_hooks")
        mod._hook = None
        mod.set_axon_ntff_profile_hook = lambda h: setattr(mod, "_hook", h)
        mod.get_axon_ntff_profile_hook = lambda: mod._hook
        sys.modules["antenv.axon_hooks"] = mod
        try:
            import antenv

            antenv.axon_hooks = mod
        except Exception:
            pass


def _run(nc, in_maps):
    kwargs = {}
    if TRACE:
        kwargs = {"trace": True, "tmpdir": TRACE_DIR}
    res = run_bass_kernel_spmd(nc, in_maps, core_ids=list(range(N_CORES)), **kwargs)
    global LAST_RESULT
    LAST_RESULT = res
    return res


def kernel(**inputs: np.ndarray) -> np.ndarray:
    x = np.asarray(inputs["input"], dtype=np.float32)
    b0 = float(np.asarray(inputs["b0"]).reshape(-1)[0])
    b1 = float(np.asarray(inputs["b1"]).reshape(-1)[0])
    a1 = float(np.asarray(inputs["a1"]).reshape(-1)[0])
    a = float(np.clip(a1, -1.0, 1.0))

    assert x.shape == (B_FULL, T_FULL, 1), x.shape
    _ensure_axon_hooks()
    xf = np.ascontiguousarray(x.reshape(B_FULL, T_FULL))

    if abs(a) <= 0.95:
        nc = _get_mm_program()
        l0, l1 = _mm_consts(a, b0, b1)
        x16 = xf.astype(np.float16)
        in_maps = []
        for c in range(N_CORES):
            # (b, t) -> [p = t%128, f = 32*(t//128) + b], contiguous fp16
            xc = x16[c * B_LOC : (c + 1) * B_LOC]
            xc = np.ascontiguousarray(
                xc.reshape(B_LOC, M_BLK, P).transpose(2, 1, 0)
            ).reshape(P, W)
            in_maps.append({"x": xc, "l0": l0, "l1": l1})
        res = _run(nc, in_maps)
        outs = []
        for c in range(N_CORES):
            oc = res.results[c]["out"].reshape(P, M_BLK, B_LOC)
            outs.append(oc.transpose(2, 1, 0).reshape(B_LOC, T_FULL))
        return (
            np.concatenate(outs, axis=0).astype(np.float32).reshape(B_FULL, T_FULL, 1)
        )

    # exact scan path for |a| near 1
    (nc, kfix) = _get_program(a, b0, b1)
    ramp_b, pmatT = _host_consts(a, kfix)
    in_maps = []
    for c in range(N_CORES):
        xc = xf[c * B_LOC : (c + 1) * B_LOC].reshape(128, T_SEG)
        in_maps.append({"x": xc, "ramp": ramp_b, "pmatT": pmatT})
    res = _run(nc, in_maps)
    outs = [res.results[c]["out"].reshape(B_LOC, T_FULL) for c in range(N_CORES)]
    return np.concatenate(outs, axis=0).reshape(B_FULL, T_FULL, 1)


if __name__ == "__main__":
    rng = np.random.default_rng(0)
    x = rng.standard_normal((B_FULL, T_FULL, 1)).astype(np.float32)
    out = kernel(
        input=x,
        b0=np.ones(1, np.float32),
        b1=np.zeros(1, np.float32),
        a1=np.full(1, 0.5, np.float32),
    )
    print(out.shape, out.dtype)


# revision 25
# speedup vs baseline: 1.1248x; 1.1248x over previous
"""One-pole IIR filter (DOnePole) on 8 Trainium2 NeuronCores.

Reference semantics (per batch element b, scan over time t):
    out_t = b0*x_t + s_t ;  s_{t+1} = b1*x_t + a*out_t   (a = clip(a1,-1,1))
i.e. out = x convolved with the causal kernel
    h_0 = b0,  h_k = (b0*a + b1) * a^(k-1)  for k >= 1.

Fast path (|a| <= 0.95): the kernel decays below fp16 resolution within a
couple hundred taps, so out is computed as a 255-tap truncated FIR entirely
on the tensor engine. The host re-packs each core's (32, 131072) slice
time-major as [128, 1024*32] fp16: partition p = t mod 128, free column
f = 32*m + b for time-block m = t div 128 and batch b. Then

    outT[:, (m,b)] = L0 @ xT[:, (m,b)] + L1 @ xT[:, (m-1,b)]

with host-built 128x128 Toeplitz matrices L0[j,p] = h[p-j] (p>=j) and
L1[j,p] = h[128+p-j]: two PE matmuls per 512-column PSUM chunk (fp32
accumulate), ACT/DVE copy the chunk back to fp16 SBUF, store. The
(m-1,b) operand is just the same SBUF tile shifted 32 columns, so tiles
carry a 32-column left margin (zero for the first tile: s_0 = 0). All
I/O is fp16 (the rel-err budget is 2e-2; fp16 costs ~3e-4, and for
a = 0.5 every h_k is a power of two, exact in fp16), which halves HBM
traffic - the hard per-core limit (~358 GB/s) that bounds this kernel.

Fallback (|a| > 0.95): the original fp32 tensor_tensor_scan kernel with
matmul segment stitching (exact for any a, ~2.8x slower).

Distribution: data-parallel over batch, 32 rows per core, both paths.
"""

import sys
from contextlib import ExitStack
from functools import lru_cache

import numpy as np

sys.path.insert(0, "/opt/trn_rl_repo")

import concourse.bass as bass  # noqa: E402
import concourse.tile as tile  # noqa: E402
from concourse import bacc, mybir  # noqa: E402
from concourse.bass_utils import run_bass_kernel_spmd  # noqa: E402

N_CORES = 8
B_FULL, T_FULL = 256, 131072
B_LOC = B_FULL // N_CORES          # 32 batch rows per core
P = 128                            # time-block size = SBUF partitions
M_BLK = T_FULL // P                # 1024 time blocks per batch row
W = M_BLK * B_LOC                  # 32768 free columns per core
MARGIN = B_LOC                     # 32-column shift = one time block
CHUNK = 512                        # matmul moving-operand max / PSUM bank
FP32 = mybir.dt.float32
FP16 = mybir.dt.float16

# Debug knobs (used by the local test harness only; harmless defaults).
TRACE = False
TRACE_DIR = None
LAST_RESULT = None


def _h_coeffs(a: float, b0: float, b1: float, n: int) -> np.ndarray:
    """Impulse response h_0..h_{n-1} of the filter, float64."""
    h = np.zeros(n, dtype=np.float64)
    h[0] = b0
    if n > 1:
        c = b0 * a + b1
        h[1:] = c * np.float64(a) ** np.arange(n - 1, dtype=np.float64)
    return h


def _mm_consts(a: float, b0: float, b1: float):
    """L0[j,p] = h[p-j] (p>=j), L1[j,p] = h[128+p-j]; fp16 [128,128]."""
    h = _h_coeffs(a, b0, b1, 2 * P)
    j = np.arange(P)[:, None]
    p = np.arange(P)[None, :]
    l0 = np.where(p >= j, h[np.minimum(p - j, 2 * P - 1)], 0.0)
    l1 = h[128 + p - j]
    return (
        np.ascontiguousarray(l0.astype(np.float16)),
        np.ascontiguousarray(l1.astype(np.float16)),
    )


def _mm_load_widths():
    """Graduated load slices: small first loads cut time-to-first-matmul;
    later loads are big so their DMA descriptors (per-partition bytes) stay
    large - SDMA round-robins between the load and store queues at PACKET
    granularity, so small descriptors starve against 8KB store descriptors.
    Multiples of 2048; sum = W."""
    return [1024, 1024, 2048, 4096, 4096, 8192, 8192, 4096]


STORE_W = 4096                     # uniform 1MB store slices


def _build_mm_program():
    """Truncated-FIR path: the filter is folded into host-built matrices,
    so one compiled program serves every (a, b0, b1) on this path.

    All SBUF buffers are statically allocated (input 8.5MB + output 8.4MB
    fit simultaneously), so no stage ever stalls on buffer reuse: loads all
    issue up front, stores stream out as soon as each 4096-col slice is
    copied from PSUM."""
    lws = _mm_load_widths()
    assert sum(lws) == W
    l_offs = np.concatenate([[0], np.cumsum(lws)]).astype(int)

    nc = bacc.Bacc("TRN2", target_bir_lowering=False, debug=False)

    x = nc.dram_tensor("x", [P, W], FP16, kind="ExternalInput")
    l0 = nc.dram_tensor("l0", [P, P], FP16, kind="ExternalInput")
    l1 = nc.dram_tensor("l1", [P, P], FP16, kind="ExternalInput")
    out = nc.dram_tensor("out", [P, W], FP16, kind="ExternalOutput")

    with tile.TileContext(nc) as tc, ExitStack() as ctx:
        wpool = ctx.enter_context(tc.tile_pool(name="wpool", bufs=1))
        xpool = ctx.enter_context(tc.tile_pool(name="xpool", bufs=1))
        opool = ctx.enter_context(tc.tile_pool(name="opool", bufs=1))
        pspool = ctx.enter_context(tc.tile_pool(name="pspool", bufs=8, space="PSUM"))

        # weights ride the (otherwise idle at t=0) ACT ring
        l0_sb = wpool.tile([P, P], FP16)
        nc.scalar.dma_start(l0_sb[:], l0[:])
        l1_sb = wpool.tile([P, P], FP16)
        nc.scalar.dma_start(l1_sb[:], l1[:])

        # PE warm-up: the HAM clock gate holds the PE at 1.2 GHz until it has
        # seen a ~3.4us busy window. The PE is otherwise idle while the first
        # loads run, so burn that head on dummy matmuls over scratch SBUF -
        # the real matmuls then start at 2.4 GHz instead of ramping (the cold
        # ramp is also the main source of run-to-run variance).
        scr = wpool.tile([P, CHUNK], FP16)
        nc.vector.memset(scr[:], 0.0)
        ps_warm = pspool.tile([P, CHUNK], FP32, tag="ps", name="ps_warm")
        for k in range(4):
            nc.tensor.matmul(
                ps_warm[:], scr[:, 0:P], scr[:], start=True, stop=True
            )

        # all input tiles (one per load slice, no reuse), loads back-to-back
        xts = []
        for i, w in enumerate(lws):
            lo = int(l_offs[i])
            xt = xpool.tile([P, MARGIN + w], FP16, tag=f"xt_{i}")
            if i == 0:
                nc.vector.memset(xt[:, 0:MARGIN], 0.0)
                nc.sync.dma_start(xt[:, MARGIN : MARGIN + w], x[:, 0:w])
            else:
                nc.sync.dma_start(xt[:, 0 : MARGIN + w], x[:, lo - MARGIN : lo + w])
            xts.append(xt)

        # store slices: uniform 1MB, tapered at the end so the final
        # copy+store tail is short
        store_ws = [4096] * 7 + [2048, 1024, 1024]
        assert sum(store_ws) == W
        grp = 0
        slo = 0
        for s, sw in enumerate(store_ws):
            ot = opool.tile([P, sw], FP16, tag=f"ot_{s}")
            # weight-groups: runs of up to 4 chunks within one load tile
            # share one LDWEIGHTS per matrix (PE reloads weights per matmul
            # group otherwise, ~100ns each)
            g = slo
            while g < slo + sw:
                ti = int(np.searchsorted(l_offs, g, side="right")) - 1
                run_end = min(slo + sw, int(l_offs[ti + 1]), g + 4 * CHUNK)
                loc = g - int(l_offs[ti])
                xt = xts[ti]
                n_ch = (run_end - g) // CHUNK
                pss = [
                    pspool.tile([P, CHUNK], FP32, tag="ps", name=f"ps_{grp}_{c}")
                    for c in range(n_ch)
                ]
                for c in range(n_ch):
                    nc.tensor.matmul(
                        pss[c][:],
                        l0_sb[:],
                        xt[:, MARGIN + loc + c * CHUNK : MARGIN + loc + (c + 1) * CHUNK],
                        start=True,
                        stop=False,
                    )
                for c in range(n_ch):
                    nc.tensor.matmul(
                        pss[c][:],
                        l1_sb[:],
                        xt[:, loc + c * CHUNK : loc + (c + 1) * CHUNK],
                        start=False,
                        stop=True,
                    )
                for c in range(n_ch):
                    osl = ot[:, g - slo + c * CHUNK : g - slo + (c + 1) * CHUNK]
                    if grp % 2 == 0:
                        nc.scalar.copy(osl, pss[c][:])
                    else:
                        nc.vector.tensor_copy(out=osl, in_=pss[c][:])
                    grp += 1
                g = run_end
            # split stores across both HWDGE rings: the sync ring is idle
            # once loads are dispatched, and a single store queue only
            # sustains ~260 GB/s
            seng = nc.scalar if s % 2 == 0 else nc.sync
            seng.dma_start(out[:, slo : slo + sw], ot[:])
            slo += sw

    nc.compile()
    return nc


# ---------------------------------------------------------------------------
# Fallback path (|a| > 0.95): fp32 tensor_tensor_scan + matmul stitching.
# ---------------------------------------------------------------------------

SEGS = 128 // B_LOC                # 4 time segments per batch row
T_SEG = T_FULL // SEGS             # 32768 columns per partition row


def _kfix(a: float) -> int:
    """Columns over which the a^t segment-stitch correction is applied."""
    aa = abs(a)
    if aa >= 1.0:
        return T_SEG
    if aa == 0.0:
        return 1
    return int(min(T_SEG, max(1, int(np.ceil(np.log(1e-14) / np.log(aa))))))


def _tile_widths():
    return [512, 1536, 2048, 4096, 4096, 4096, 4096, 4096, 4096, 2048, 2048]


def _build_program(a: float, b0: float, b1: float, kfix: int):
    widths = _tile_widths()
    offs = np.concatenate([[0], np.cumsum(widths)]).astype(int)
    n_t = len(widths)
    held_idx = [i for i in range(n_t) if offs[i] < kfix]

    nc = bacc.Bacc("TRN2", target_bir_lowering=False, debug=False)

    x = nc.dram_tensor("x", [128, T_SEG], FP32, kind="ExternalInput")
    ramp = nc.dram_tensor("ramp", [128, kfix], FP32, kind="ExternalInput")
    pmatT = nc.dram_tensor("pmatT", [128, 128], FP32, kind="ExternalInput")
    out = nc.dram_tensor("out", [128, T_SEG], FP32, kind="ExternalOutput")

    fast = (b1 == 0.0)
    ident = fast and (b0 == 1.0)

    with tile.TileContext(nc) as tc, ExitStack() as ctx:
        many_held = len(held_idx) > 8
        cpool = ctx.enter_context(tc.tile_pool(name="cpool", bufs=1))
        xpool = ctx.enter_context(
            tc.tile_pool(name="xpool", bufs=2 if many_held else 4)
        )
        hpool = ctx.enter_context(tc.tile_pool(name="hpool", bufs=1))
        spool = ctx.enter_context(tc.tile_pool(name="spool", bufs=1))
        pspool = ctx.enter_context(tc.tile_pool(name="pspool", bufs=1, space="PSUM"))
        opool = ctx.enter_context(
            tc.tile_pool(name="opool", bufs=2 if many_held else 3)
        )

        ac2k = cpool.tile([128, 2048], FP32)
        nc.gpsimd.memset(ac2k[:], a)
        ac4k = cpool.tile([128, 4096], FP32)
        nc.gpsimd.memset(ac4k[:], a)

        if not ident:
            zcol = spool.tile([128, 1], FP32)
            nc.gpsimd.memset(zcol[:], 0.0)

        held = {}
        prev_out = None
        prev_x = None
        prev_w = 0
        for i in range(n_t):
            w = widths[i]
            lo, hi = int(offs[i]), int(offs[i] + w)
            wide = w > 2048
            xt = xpool.tile(
                [128, 4096 if wide else 2048], FP32,
                tag="xtb" if wide else "xt", bufs=3 if wide else None,
                name=f"xt_{i}",
            )
            nc.sync.dma_start(xt[:, 0:w], x[:, lo:hi])

            if ident:
                data1 = xt
            else:
                ut = xpool.tile(
                    [128, 4096 if wide else 2048], FP32,
                    tag="utb" if wide else "ut", bufs=3 if wide else None,
                    name=f"ut_{i}",
                )
                nc.scalar.mul(ut[:, 0:w], xt[:, 0:w], b0)
                if not fast:
                    nc.vector.scalar_tensor_tensor(
                        out=ut[:, 1:w],
                        in0=xt[:, 0 : w - 1],
                        scalar=b1,
                        in1=ut[:, 1:w],
                        op0=mybir.AluOpType.mult,
                        op1=mybir.AluOpType.add,
                    )
                    xprev_col = (
                        zcol[:, 0:1] if i == 0 else prev_x[:, prev_w - 1 : prev_w]
                    )
                    nc.vector.scalar_tensor_tensor(
                        out=ut[:, 0:1],
                        in0=xprev_col,
                        scalar=b1,
                        in1=ut[:, 0:1],
                        op0=mybir.AluOpType.mult,
                        op1=mybir.AluOpType.add,
                    )
                data1 = ut

            is_held = i in held_idx
            ot = (hpool if is_held else opool).tile(
                [128, 4096 if wide else 2048], FP32,
                tag=(f"held{i}" if is_held else ("otb" if wide else "ot")),
                bufs=1 if is_held else None, name=f"ot_{i}",
            )
            init = 0.0 if i == 0 else prev_out[:, prev_w - 1 : prev_w]
            ac = ac4k if wide else ac2k
            nc.vector.tensor_tensor_scan(
                out=ot[:, 0:w],
                data0=ac[:, 0:w],
                data1=data1[:, 0:w],
                initial=init,
                op0=mybir.AluOpType.mult,
                op1=mybir.AluOpType.add,
            )
            if is_held:
                held[i] = ot
            else:
                seng = nc.sync if i >= n_t - 3 else nc.scalar
                seng.dma_start(out[:, lo:hi], ot[:, 0:w])
            prev_out = ot
            prev_x = xt
            prev_w = w

        pm_sb = spool.tile([128, 128], FP32)
        nc.scalar.dma_start(pm_sb[:], pmatT[:])

        d_t = spool.tile([128, 1], FP32)
        nc.vector.tensor_scalar_mul(d_t[:], prev_out[:, prev_w - 1 : prev_w], a)
        if not fast:
            nc.vector.scalar_tensor_tensor(
                out=d_t[:],
                in0=prev_x[:, prev_w - 1 : prev_w],
                scalar=b1,
                in1=d_t[:],
                op0=mybir.AluOpType.mult,
                op1=mybir.AluOpType.add,
            )

        s_ps = pspool.tile([128, 1], FP32)
        nc.tensor.matmul(s_ps[:], pm_sb[:], d_t[:], start=True, stop=True)
        s_sb = spool.tile([128, 1], FP32)
        nc.scalar.copy(s_sb[:], s_ps[:])

        for i in held_idx:
            ot = held[i]
            lo = int(offs[i])
            w = min(widths[i], kfix - lo)
            rt = xpool.tile(
                [128, min(kfix, widths[i])], FP32, tag="rt",
                bufs=1 if many_held else 2, name=f"rt_{i}",
            )
            nc.scalar.dma_start(rt[:, 0:w], ramp[:, lo : lo + w])
            nc.vector.scalar_tensor_tensor(
                out=ot[:, 0:w],
                in0=rt[:, 0:w],
                scalar=s_sb[:],
                in1=ot[:, 0:w],
                op0=mybir.AluOpType.mult,
                op1=mybir.AluOpType.add,
            )
            nc.sync.dma_start(out[:, lo : lo + widths[i]], ot[:, 0 : widths[i]])

    nc.compile()
    return nc


@lru_cache(maxsize=8)
def _get_mm_program():
    return _build_mm_program()


@lru_cache(maxsize=8)
def _get_program(a: float, b0: float, b1: float):
    """Program used for profiling hooks in the local harness: returns the
    matmul-path program when it applies, else the scan-path program."""
    if abs(a) <= 0.95:
        return _get_mm_program(), 0
    kfix = _kfix(a)
    return _build_program(a, b0, b1, kfix), kfix


def _host_consts(a: float, kfix: int):
    ramp = (np.float64(a) ** np.arange(kfix, dtype=np.float64)).astype(np.float32)
    ramp_b = np.ascontiguousarray(np.broadcast_to(ramp[None, :], (128, kfix)))
    aL = np.float64(a) ** np.float64(T_SEG)
    Pm = np.zeros((128, 128), dtype=np.float64)
    for b in range(B_LOC):
        for j in range(SEGS):
            for j2 in range(j):
                Pm[SEGS * b + j, SEGS * b + j2] = aL ** (j - j2 - 1)
    pmatT = np.ascontiguousarray(Pm.T.astype(np.float32))
    return ramp_b, pmatT


def _ensure_axon_hooks():
    """bass_utils imports antenv.axon_hooks when tracing is requested; some
    images lack that module. Provide a stub that reports 'no hook' so
    execution proceeds untraced instead of dying."""
    try:
        import antenv.axon_hooks  # noqa: F401
    except Exception:
        import types

        mod = types.ModuleType("antenv.axon_hooks")
        mod._hook = None
        mod.set_axon_ntff_profile_hook = lambda h: setattr(mod, "_hook", h)
        mod.get_axon_ntff_profile_hook = lambda: mod._hook
        sys.modules["antenv.axon_hooks"] = mod
        try:
            import antenv

            antenv.axon_hooks = mod
        except Exception:
            pass


def _run(nc, in_maps):
    kwargs = {}
    if TRACE:
        kwargs = {"trace": True, "tmpdir": TRACE_DIR}
    res = run_bass_kernel_spmd(nc, in_maps, core_ids=list(range(N_CORES)), **kwargs)
    global LAST_RESULT
    LAST_RESULT = res
    return res


def kernel(**inputs: np.ndarray) -> np.ndarray:
    x = np.asarray(inputs["input"], dtype=np.float32)
    b0 = float(np.asarray(inputs["b0"]).reshape(-1)[0])
    b1 = float(np.asarray(inputs["b1"]).reshape(-1)[0])
    a1 = float(np.asarray(inputs["a1"]).reshape(-1)[0])
    a = float(np.clip(a1, -1.0, 1.0))

    assert x.shape == (B_FULL, T_FULL, 1), x.shape
    _ensure_axon_hooks()
    xf = np.ascontiguousarray(x.reshape(B_FULL, T_FULL))

    if abs(a) <= 0.95:
        nc = _get_mm_program()
        l0, l1 = _mm_consts(a, b0, b1)
        x16 = xf.astype(np.float16)
        in_maps = []
        for c in range(N_CORES):
            # (b, t) -> [p = t%128, f = 32*(t//128) + b], contiguous fp16
            xc = x16[c * B_LOC : (c + 1) * B_LOC]
            xc = np.ascontiguousarray(
                xc.reshape(B_LOC, M_BLK, P).transpose(2, 1, 0)
            ).reshape(P, W)
            in_maps.append({"x": xc, "l0": l0, "l1": l1})
        res = _run(nc, in_maps)
        outs = []
        for c in range(N_CORES):
            oc = res.results[c]["out"].reshape(P, M_BLK, B_LOC)
            outs.append(oc.transpose(2, 1, 0).reshape(B_LOC, T_FULL))
        return (
            np.concatenate(outs, axis=0).astype(np.float32).reshape(B_FULL, T_FULL, 1)
        )

    # exact scan path for |a| near 1
    (nc, kfix) = _get_program(a, b0, b1)
    ramp_b, pmatT = _host_consts(a, kfix)
    in_maps = []
    for c in range(N_CORES):
        xc = xf[c * B_LOC : (c + 1) * B_LOC].reshape(128, T_SEG)
        in_maps.append({"x": xc, "ramp": ramp_b, "pmatT": pmatT})
    res = _run(nc, in_maps)
    outs = [res.results[c]["out"].reshape(B_LOC, T_FULL) for c in range(N_CORES)]
    return np.concatenate(outs, axis=0).reshape(B_FULL, T_FULL, 1)


if __name__ == "__main__":
    rng = np.random.default_rng(0)
    x = rng.standard_normal((B_FULL, T_FULL, 1)).astype(np.float32)
    out = kernel(
        input=x,
        b0=np.ones(1, np.float32),
        b1=np.zeros(1, np.float32),
        a1=np.full(1, 0.5, np.float32),
    )
    print(out.shape, out.dtype)
